# revision 16
# baseline (speedup 1.0000x reference)
"""Trainium2 Bass kernel: autoregressive LSTM decoder (nn_DecoderAR).

Reference computation (per step t over HORIZON=24):
    inp   = concat([x_t, y_prev])                      (B, 8)
    gates = inp @ W_ih.T + b_ih + h @ W_hh.T + b_hh    (B, 2048)
    i, f, g, o = split(gates); sigmoid/tanh
    c = f*c + i*g ; h = o*tanh(c)
    logit = h @ fc_w.T + fc_b ; y_prev = sigmoid(logit)
Output: logits (B, 24, 1).

Sharding: data-parallel over batch (8192 -> 8 cores x 1024), weights
replicated.  On-chip layout keeps batch on the matmul free dim and
gate/hid dims on partitions, so the recurrence needs no transposes.

Mixed precision (validated numerically against the fp32 reference;
rel err ~1.56e-2 < 2e-2 gate):
  - i/f/o gate matmuls (hh + extras) run fp8e4m3 with
    perf_mode=DoubleRow (two 128-row K-planes per instruction at 0.5
    cyc/row) on an fp8 copy of h; their sigmoids saturate, so the ~3%
    pre-activation error is tolerated.
  - The g (candidate) gate feeds c directly, so its hh term stays
    fp16; its extras (9 K-rows, no sqrt(K) error growth) are fp8 DR.
  - gate biases are folded into the extras matmul as ones-rhs rows
    (split hi/lo for fp8), so each [i,f,o] PSUM tile drains with ONE
    merged sigmoid (N=1536), amortizing the ~350-cycle ACT fill.

The engine balance (timeline-sim): ACT is the wall (~10.5us per
512-col half-step: 6144 sigmoid + 2048 tanh-g + 2048 tanh-c FD
cycles at 1 elem/lane/cycle); PE ~8.6us, DVE ~5.7us, Pool ~2.4us.
Everything else is scheduled to keep ACT saturated:
  - two batch halves interleaved so one half's ACT/DVE tail hides
    under the other half's matmuls;
  - the fc/y tail of a half-step is emitted in the MIDDLE of the
    other half's chains: late enough that h is ready (no PE stall),
    early enough that y feeds the next step's extras matmul;
  - tanh(c) is split in two 2-chunk pieces so the first piece fills
    the ACT bubble while the vector engine finishes the last c chunk;
  - the last step's h8 cast (dead state) is skipped.
(A transposed-fc variant that shrinks the y sigmoid to FD=128 exists
behind fc_t=True but is rejected by the BIR verifier: engine access
patterns cannot stride the partition dim.)
"""

import numpy as np
import ml_dtypes

import concourse.bass as bass
import concourse.mybir as mybir
import concourse.tile as tile
from concourse import bacc
from concourse.bass_utils import run_bass_kernel_spmd

B, HORIZON, NCOV, HID = 8192, 24, 7, 512
NCORES = 8
BL = B // NCORES          # batch rows per core (1024)
P = 128
KC = HID // P             # hid chunks (4)
KE = NCOV + 2             # extras rows: y, 7 covariates, ones (bias)

F32 = mybir.dt.float32
F16 = mybir.dt.float16
F8 = mybir.dt.float8e4
AF = mybir.ActivationFunctionType
DR = mybir.MatmulPerfMode.DoubleRow
F16NP = np.float16
F8NP = ml_dtypes.float8_e4m3

# i/f/o = DoubleRow fp8 gates; their PyTorch gate indices
DR_GATES = (0, 1, 3)
MID_J = 3
PACK_EXTRAS = False  # legacy flag, kept for test.py compatibility


def build_program(horizon=HORIZON, bl=BL, repeats=1, pack=None, fc_t=False):
    """Build the single-core Bass program (identical on all cores).

    repeats>1 re-runs the whole recurrence (benchmarking only: the extra
    passes reuse the same x slots / output rows, so results are those of
    the last pass, which no longer matches the reference)."""
    bh = bl // 2  # batch half = matmul free dim (512)
    nc = bacc.Bacc(None)

    # fp8 extras operand: 2 K-planes of 5 rows:
    # plane0 = [y, x0..x3], plane1 = [x4..x6, ones, ones]
    xrep8 = nc.declare_dram_parameter("xrep8", [5, horizon, 2, bl], F8, False)
    h016 = nc.declare_dram_parameter("h016", [P, KC, bl], F16, False)
    h08 = nc.declare_dram_parameter("h08", [P, KC, bl], F8, False)
    c016 = nc.declare_dram_parameter("c016", [P, KC, bl], F16, False)
    whh8 = nc.declare_dram_parameter("whh8", [P, KC, 3 * HID], F8, False)
    whh16 = nc.declare_dram_parameter("whh16", [P, KC, HID], F16, False)
    # fp8 extras weights, jb-ordered (ifo: jb=j*3+gi; g: 3*KC+j); plane1
    # rows 3/4 hold the bias split hi/lo (ones rhs)
    we8 = nc.declare_dram_parameter("we8", [5, 4 * KC, 2, P], F8, False)
    fcw = nc.declare_dram_parameter("fcw", [P, KC], F16, False)
    fcb = nc.declare_dram_parameter("fcb", [P, 1], F32, False)
    out = nc.declare_dram_parameter("out", [horizon, bl], F32, True)

    with tile.TileContext(nc) as tc:
        with (
            tc.tile_pool(name="singles", bufs=1) as singles,
            tc.tile_pool(name="gacts", bufs=4) as gacts,
            tc.tile_pool(name="tails", bufs=4) as tails,
            tc.tile_pool(name="ps_ifo", bufs=2, space="PSUM") as ps_ifo,
            tc.tile_pool(name="ps_g", bufs=1, space="PSUM") as ps_g,
            tc.tile_pool(name="ps_fc", bufs=1, space="PSUM") as ps_fc,
        ):
            # --- resident tensors; DMA order matters: the first-step
            # matmuls need the j=0 whh8 blocks + half-0 h8/xs8 first ---
            whh8_sb = singles.tile([P, KC, 3 * HID], F8, tag="whh8", name="whh8")
            nc.sync.dma_start(whh8_sb[:, :, 0:3 * P], whh8[:, :, 0:3 * P])
            xs8_sb = []
            h16_sb = [[singles.tile([P, KC, bh], F16, tag=f"h16_{hf}_{i}", name=f"h16_{hf}_{i}")
                       for i in range(2)] for hf in range(2)]
            h8_sb = [[singles.tile([P, KC, bh], F8, tag=f"h8_{hf}_{i}", name=f"h8_{hf}_{i}")
                      for i in range(2)] for hf in range(2)]
            c_sb = [[singles.tile([P, KC, bh], F16, tag=f"c_{hf}_{i}", name=f"c_{hf}_{i}")
                     for i in range(2)] for hf in range(2)]
            for hf in range(2):
                x8 = singles.tile([5, horizon, 2, bh], F8, tag=f"xrep8{hf}", name=f"xrep8{hf}")
                xs8_sb.append(x8)
            we8_sb = singles.tile([5, 4 * KC, 2, P], F8, tag="we8", name="we8")
            whh16_sb = singles.tile([P, KC, HID], F16, tag="whh16", name="whh16")
            fcw_sb = singles.tile([P, KC], F16, tag="fcw", name="fcw")
            fcb_sb = singles.tile([P, 1], F32, tag="fcb", name="fcb")
            nc.sync.dma_start(h8_sb[0][0][:], h08[:, :, 0:bh])
            nc.sync.dma_start(xs8_sb[0][:], xrep8[:, :, :, 0:bh])
            nc.sync.dma_start(we8_sb[:], we8[:])
            nc.sync.dma_start(whh16_sb[:, :, 0:P], whh16[:, :, 0:P])
            nc.sync.dma_start(h16_sb[0][0][:], h016[:, :, 0:bh])
            nc.sync.dma_start(c_sb[0][0][:], c016[:, :, 0:bh])
            nc.sync.dma_start(whh8_sb[:, :, 3 * P:], whh8[:, :, 3 * P:])
            nc.sync.dma_start(whh16_sb[:, :, P:], whh16[:, :, P:])
            nc.sync.dma_start(h8_sb[1][0][:], h08[:, :, bh:])
            nc.sync.dma_start(xs8_sb[1][:], xrep8[:, :, :, bh:])
            nc.sync.dma_start(h16_sb[1][0][:], h016[:, :, bh:])
            nc.sync.dma_start(c_sb[1][0][:], c016[:, :, bh:])
            nc.sync.dma_start(fcw_sb[:], fcw[:])
            nc.sync.dma_start(fcb_sb[:], fcb[:])

            def emit_chains(hf, t, tg, mid=None):
                """Gate chains + c/h updates for one (half, step); calls
                mid() after the MID_J group (late enough that the other
                half's h is ready, early enough for its y deadline)."""
                cur, nxt = tg % 2, (tg + 1) % 2
                xs8 = xs8_sb[hf]
                h16c, h16n = h16_sb[hf][cur], h16_sb[hf][nxt]
                h8c, h8n = h8_sb[hf][cur], h8_sb[hf][nxt]
                ccur, cnxt = c_sb[hf][cur], c_sb[hf][nxt]
                g3s = []
                for j in range(KC):
                    ps3 = ps_ifo.tile([P, 3, bh], F32, tag="ps3", name="ps3")
                    for gi, gk in enumerate(DR_GATES):
                        jb = j * 3 + gi
                        nc.tensor.matmul(
                            ps3[:, gi, :],
                            whh8_sb[:, 0:2, jb * P:(jb + 1) * P],
                            h8c[:, 0:2, :],
                            start=True, stop=False, perf_mode=DR,
                        )
                        nc.tensor.matmul(
                            ps3[:, gi, :],
                            whh8_sb[:, 2:4, jb * P:(jb + 1) * P],
                            h8c[:, 2:4, :],
                            start=False, stop=False, perf_mode=DR,
                        )
                        # extras last: y of this step is produced late
                        nc.tensor.matmul(
                            ps3[:, gi, :], we8_sb[:, jb, :, :], xs8[:, t, :, :],
                            start=False, stop=True, perf_mode=DR,
                        )
                    psg = ps_g.tile([P, bh], F32, tag="psg", name="psg")
                    for kk in range(KC):
                        nc.tensor.matmul(
                            psg[:],
                            whh16_sb[:, kk, j * P:(j + 1) * P],
                            h16c[:, kk, :],
                            start=(kk == 0), stop=False,
                        )
                    nc.tensor.matmul(
                        psg[:], we8_sb[:, 3 * KC + j, :, :], xs8[:, t, :, :],
                        start=False, stop=True, perf_mode=DR,
                    )
                    g3 = gacts.tile([P, 3, bh], F16, tag="g3", name="g3")
                    nc.scalar.activation(g3[:], ps3[:], AF.Sigmoid)
                    gg = gacts.tile([P, bh], F16, tag="gg", name="gg")
                    nc.scalar.activation(gg[:], psg[:], AF.Tanh)
                    t1 = tails.tile([P, bh], F16, tag="t1", name="t1")
                    t2 = tails.tile([P, bh], F16, tag="t2", name="t2")
                    nc.vector.tensor_mul(t1[:], gg[:], g3[:, 0, :])
                    nc.vector.tensor_mul(t2[:], g3[:, 1, :], ccur[:, j, :])
                    nc.vector.tensor_add(cnxt[:, j, :], t1[:], t2[:])
                    g3s.append(g3)
                    if j == MID_J and mid is not None:
                        mid()
                # tanh over c in 2-chunk pieces: the first piece fills the
                # ACT bubble while DVE finishes the j=3 c chunk; h muls
                # follow per piece so fc/h8 start earlier.  The h8 cast is
                # one big op on the (otherwise idle) gpsimd.
                tnh = tails.tile([P, KC, bh], F16, tag="tnh", name="tnh")
                for piece in range(2):
                    pc = slice(2 * piece, 2 * piece + 2)
                    nc.scalar.activation(tnh[:, pc, :], cnxt[:, pc, :], AF.Tanh)
                    for j in (2 * piece, 2 * piece + 1):
                        nc.vector.tensor_mul(h16n[:, j, :], g3s[j][:, 2, :],
                                             tnh[:, j, :])
                if tg + 1 < horizon * repeats:  # last step's h8 is dead
                    nc.gpsimd.tensor_copy(h8n[:], h16n[:])

            def emit_tail(hf, t, tg):
                """fc logit + y recirculation for one (half, step).

                Transposed fc: per batch-chunk bc, lhsT = h16 chunk
                (stationary), rhs = fc_w column -> [128,1] output placed
                at PSUM partition bc*32... no: partitions 0/32/64/96
                (the only legal engine write starts), column bc."""
                nxt = (tg + 1) % 2
                h16n = h16_sb[hf][nxt]
                if fc_t:
                    # [1,128] logit rows land at PSUM partitions 0/32/64/96
                    # (the legal engine-write starts); kk outer keeps the
                    # fcw stationary constant across the 4 bc matmuls.
                    psfc = ps_fc.tile([P, P], F32, tag="fc", name="fc")
                    for kk in range(KC):
                        for bc in range(KC):
                            nc.tensor.matmul(
                                psfc[bc * 32:bc * 32 + 1, :],
                                fcw_sb[:, kk:kk + 1],
                                h16n[:, kk, bc * P:(bc + 1) * P],
                                start=(kk == 0), stop=(kk == KC - 1),
                                tile_position=(0, bc * 32),
                            )
                    # strided-partition view [4@32, 128] drains in one op
                    pv = psfc[0:97:32, :]
                    if tg + 1 < horizon * repeats:
                        tn = (t + 1) % horizon
                        y16 = tails.tile([P, P], F16, tag="y16", name="y16")
                        nc.scalar.activation(y16[0:97:32, :], pv,
                                             AF.Sigmoid, bias=fcb_sb[0:97:32, :])
                        y8 = tails.tile([P, P], F8, tag="y8", name="y8")
                        nc.vector.tensor_copy(y8[0:97:32, :], y16[0:97:32, :])
                        nc.sync.dma_start(
                            xs8_sb[hf][0:1, tn, 0:1, :], y8[0:97:32, :])
                    osl = tails.tile([P, P], F32, tag="osl", name="osl")
                    nc.vector.tensor_scalar_add(osl[0:97:32, :], pv, fcb_sb[0:97:32, :])
                    nc.sync.dma_start(
                        out[t:t + 1, hf * bh:(hf + 1) * bh], osl[0:97:32, :])
                else:
                    fc_ps = ps_fc.tile([1, bh], F32, tag="fc", name="fc")
                    for j in range(KC):
                        nc.tensor.matmul(
                            fc_ps[:], fcw_sb[:, j:j + 1], h16n[:, j, :],
                            start=(j == 0), stop=(j == KC - 1),
                        )
                    if tg + 1 < horizon * repeats:
                        tn = (t + 1) % horizon
                        y16 = tails.tile([1, bh], F16, tag="y16", name="y16")
                        nc.scalar.activation(y16[:], fc_ps[:],
                                             AF.Sigmoid, bias=fcb_sb[0:1, :])
                        nc.vector.tensor_copy(xs8_sb[hf][0:1, tn, 0:1, :], y16[:])
                    osl = tails.tile([1, bh], F32, tag="osl", name="osl")
                    nc.vector.tensor_scalar_add(osl[:], fc_ps[:], fcb_sb[0:1, :])
                    nc.sync.dma_start(out[t:t + 1, hf * bh:(hf + 1) * bh], osl[:])

            pending = [None]

            def mid():
                if pending[0] is not None:
                    emit_tail(*pending[0])
                    pending[0] = None

            for rep in range(repeats):
                for t in range(horizon):
                    tg = rep * horizon + t
                    for hf in range(2):
                        emit_chains(hf, t, tg, mid=mid)
                        pending[0] = (hf, t, tg)
            emit_tail(*pending[0])

    nc.finalize()
    return nc


def prepare_inputs(future_x, h_enc, c_enc, y0, W_ih, W_hh, b_ih, b_hh,
                   fc_w, fc_b, horizon=HORIZON, bl=BL, ncores=NCORES,
                   pack=None):
    """Host-side shard + layout prep. Returns list of per-core input dicts."""
    future_x = np.asarray(future_x, np.float32)
    h_enc = np.asarray(h_enc, np.float32)
    c_enc = np.asarray(c_enc, np.float32)
    y0 = np.asarray(y0, np.float32)
    W_ih = np.asarray(W_ih, np.float32)
    W_hh = np.asarray(W_hh, np.float32)
    bias = np.asarray(b_ih, np.float32) + np.asarray(b_hh, np.float32)
    fc_w = np.asarray(fc_w, np.float32)
    fc_b = np.asarray(fc_b, np.float32)

    # --- replicated weights ---
    # whh8: i/f/o gate chunks, fp8, column blocks ordered (j, gi)
    # whh8[p, k, (j*3+gi)*128 + m] = W_hh[gk*512 + j*128 + m, k*128 + p]
    Wt = W_hh.T.reshape(KC, P, 4 * HID)        # [k, p, row]
    whh8_host = np.empty((P, KC, 3 * HID), F8NP)
    whh16_host = np.empty((P, KC, HID), F16NP)
    for j in range(KC):
        for gi, gk in enumerate(DR_GATES):
            rows = slice(gk * HID + j * P, gk * HID + (j + 1) * P)
            jb = j * 3 + gi
            whh8_host[:, :, jb * P:(jb + 1) * P] = (
                Wt[:, :, rows].transpose(1, 0, 2).astype(F8NP))
        rows = slice(2 * HID + j * P, 2 * HID + (j + 1) * P)
        whh16_host[:, :, j * P:(j + 1) * P] = (
            Wt[:, :, rows].transpose(1, 0, 2).astype(F16NP))

    # we8: fp8 extras weights (all 4 gates); planes of 5 rows:
    #   plane0 = [wy, wx0..wx3], plane1 = [wx4..wx6, b_hi, b_lo]
    # block order: ifo jb = j*3+gi, g at 3*KC + j
    wih_r = W_ih.reshape(4 * KC, P, NCOV + 1)
    bias_r = bias.reshape(4 * KC, P)
    b_hi = bias_r.astype(F8NP).astype(np.float32)
    b_lo = (bias_r - b_hi).astype(F8NP)
    we8_host = np.zeros((5, 4 * KC, 2, P), F8NP)

    def fill_we8(jb, mc):
        we8_host[0, jb, 0] = wih_r[mc, :, NCOV].astype(F8NP)
        for cv in range(4):
            we8_host[1 + cv, jb, 0] = wih_r[mc, :, cv].astype(F8NP)
        for cv in range(4, NCOV):
            we8_host[cv - 4, jb, 1] = wih_r[mc, :, cv].astype(F8NP)
        we8_host[3, jb, 1] = b_hi[mc].astype(F8NP)
        we8_host[4, jb, 1] = b_lo[mc]

    for j in range(KC):
        for gi, gk in enumerate(DR_GATES):
            fill_we8(j * 3 + gi, gk * KC + j)
        fill_we8(3 * KC + j, 2 * KC + j)

    fcw_host = np.ascontiguousarray(fc_w.reshape(KC, P).T).astype(F16NP)
    fcb_host = np.full((P, 1), float(fc_b[0]), np.float32)

    in_maps = []
    for core in range(ncores):
        sl = slice(core * bl, (core + 1) * bl)
        # fp8 extras rhs: plane0 = [y, x0..x3], plane1 = [x4..x6, 1, 1]
        xt8 = future_x[sl, :horizon].transpose(2, 1, 0).astype(F8NP)
        xrep8_host = np.zeros((5, horizon, 2, bl), F8NP)
        xrep8_host[0, 0, 0] = y0[sl, 0].astype(F8NP)  # later steps on-device
        xrep8_host[1:5, :, 0] = xt8[0:4]
        xrep8_host[0:3, :, 1] = xt8[4:7]
        xrep8_host[3:5, :, 1] = 1.0
        h16_host = np.ascontiguousarray(
            h_enc[sl].T.reshape(KC, P, bl).transpose(1, 0, 2)).astype(F16NP)
        c16_host = np.ascontiguousarray(
            c_enc[sl].T.reshape(KC, P, bl).transpose(1, 0, 2)).astype(F16NP)
        in_maps.append({
            "xrep8": xrep8_host,
            "h016": h16_host,
            "h08": h16_host.astype(F8NP),
            "c016": c16_host,
            "whh8": whh8_host,
            "whh16": whh16_host,
            "we8": we8_host,
            "fcw": fcw_host,
            "fcb": fcb_host,
        })
    return in_maps


def run(inputs, trace=False, **kwargs):
    """Run on 8 NeuronCores; returns (full_output, BassKernelResults)."""
    nc = build_program()
    in_maps = prepare_inputs(**inputs)
    res = run_bass_kernel_spmd(nc, in_maps, core_ids=list(range(NCORES)),
                               trace=trace, **kwargs)
    full = np.empty((B, HORIZON, 1), np.float32)
    for core in range(NCORES):
        o = np.asarray(res.results[core]["out"], np.float32)  # (HORIZON, BL)
        full[core * BL:(core + 1) * BL, :, 0] = o.T
    return full, res


def kernel(**inputs):
    out, _ = run(inputs)
    return out
